# revision 81
# baseline (speedup 1.0000x reference)
"""Trainium2 Bass kernel for nn_BaseMultiHeadAttention (B=2, S=2048, E=1024, H=16).

Sharding: tensor-parallel over heads - each of the 8 NeuronCores handles 2
heads for both batch elements.  RMSNorm + RoPE + causal attention run
per-head on-device; the output projection is row-sharded (each core
contracts its 128 ctx features against proj_w) and the host sums the 8
fp16 partial [B*S, E] outputs (the all-reduce) and adds the bias.

Key layout/perf choices (vs the f32 baseline):
  * All device inputs are fp16 (host casts; ~5e-4 rel err, well inside the
    2e-2 gate).  PE matmuls run at 1 cyc/row at any moving size, DVE
    elementwise ops with all-fp16 operands run at 2x, transposes write fp16
    PSUM so the PSUM->SBUF copies also run at 2x.
  * Both heads are packed in the feature dim: tiles are [128 s, NT, 128]
    where 128 = 2 heads x 64 rope-permuted features; a single PE transpose
    per s-tile yields qT/kT with head h on partitions [64h, 64h+64).
  * RMSNorm: sum-of-squares is rotation-invariant, so k is roped
    unnormalized and its 1/rms (x softmax 1/sqrt(D), folded via
    sqrt(ss + D*eps)) is applied as the per-partition scale AP of the Exp
    activation - k normalization costs zero elementwise work.  q gets one
    fused scale multiply after rope.
  * Scores/softmax at [1 k-tile x 1024 q-chunk] granularity: exact causal
    trimming of both the score matmuls and the exp widths; p = exp(scores)
    unnormalized in fp16 (bounded by e^8), with the ones-column of v giving
    softmax row-sums inside the ctx matmul; 1/rowsum is fused into the
    mandatory ctx PSUM->SBUF copy.
  * Output projection partials are written as fp16 (halves the 16MB
    output DMA).  GPSIMD cannot touch PSUM, so all PSUM->SBUF copies run
    on DVE, with ACT absorbing the lead-in/tail copies where it idles;
    flexible SBUF elementwise work (sq / q-scale / causal masks) is
    spread over ACT and Pool.
  * Emission order is a software pipeline tuned against the timeline
    simulator: scores stream per k-tile (one exp per tile, ACT is the
    critical engine); ctx matmuls are queued and pumped in small bursts
    between score tiles so PE never starves ACT; projection of group N-1
    is emitted after ctx of group N so DVE output copies overlap PE ctx
    work; the first chunk is split 512-wide so exps start ~7us in.
"""
import numpy as np

import bass_rust
import concourse.bass as bass
import concourse.mybir as mybir
import concourse.tile as tile
from concourse.bass_utils import run_bass_kernel_spmd
from concourse.masks import make_identity

B, S, E, H, D = 2, 2048, 1024, 16, 64
HD = D // 2                # 32 rope pair count
N_CORES = 8
HL = H // N_CORES          # 2 heads per core
D2 = HL * D                # 128 packed feature dim
NT = S // 128              # 16 s-tiles
NCH = 2                    # q-chunks of 1024
CW = S // NCH              # 1024 chunk width
KT = S // 128              # 16 k-tiles
EPS = 1.1920928955078125e-07
f32 = mybir.dt.float32
f16 = mybir.dt.float16
ALU = mybir.AluOpType
ACTF = mybir.ActivationFunctionType

# engine-balance knobs.  GPSIMD/Pool cannot touch PSUM on real HW, so all
# PSUM->SBUF copies go to DVE (optionally some to ACT via the pattern).
DIAG_ON_POOL = True        # diagonal-tile causal zeroing of p
PUMP_K = 16                # ctx matmuls interleaved per score tile
OSB_ACT_PATTERN = (0, 0, 0, 0)  # per proj-half-tile: 1 -> ACT Copy

_TC = tile.TileContext


def _legalize_waits(nc):
    """Split multi-wait sync_infos for this walrus build (1 wait/instr)."""
    uid = 0
    for f in nc.m.functions:
        for blk in f.blocks:
            insts = list(blk.instructions)
            out, changed = [], False
            for inst in insts:
                si = inst.sync_info
                cap = 2 if isinstance(inst, mybir.InstEventSemaphore) else 1
                if si is not None and len(si.on_wait) > cap:
                    changed = True
                    waits = list(si.on_wait)
                    for w in waits[:-cap]:
                        carrier = mybir.InstNoOp(
                            name=f"legwait-{uid}", engine=inst.engine,
                            ins=[], outs=[])
                        uid += 1
                        carrier.sync_info = bass_rust.SyncInfo(
                            on_wait=[w], on_update=[])
                        nc.register_instruction(carrier, overwrite=True)
                        out.append(carrier)
                    si.on_wait = waits[-cap:]
                    inst.sync_info = si
                out.append(inst)
            if changed:
                blk.instructions = out


def _bcast(ap, count):
    """Append a step-0 (broadcast) innermost free dim of `count`."""
    return bass.AP(tensor=ap.tensor, offset=ap.offset,
                   ap=[list(d) for d in ap.ap] + [[0, count]])


def _bcast_mid(ap, count, pos):
    """Insert a step-0 broadcast dim of `count` before ap dim `pos`."""
    dims = [list(d) for d in ap.ap]
    return bass.AP(tensor=ap.tensor, offset=ap.offset,
                   ap=dims[:pos] + [[0, count]] + dims[pos:])


def build_nc():
    nc = bass.Bass("TRN2", target_bir_lowering=False, debug=False)
    q_in = nc.dram_tensor("q", [B, 128, NT, D2], f16, kind="ExternalInput")
    k_in = nc.dram_tensor("k", [B, 128, NT, D2], f16, kind="ExternalInput")
    v_in = nc.dram_tensor("v", [B, 128, NT, HL, D + 1], f16,
                          kind="ExternalInput")
    cos_in = nc.dram_tensor("cos", [128, NT, HD], f16, kind="ExternalInput")
    sin_in = nc.dram_tensor("sin", [128, NT, HD], f16, kind="ExternalInput")
    wt_in = nc.dram_tensor("wt", [128, E], f16, kind="ExternalInput")
    out = nc.dram_tensor("out", [B * S, E], f16, kind="ExternalOutput")

    with _TC(nc) as tc:
        with tc.tile_pool(name="const", bufs=1) as cp, \
             tc.tile_pool(name="pa", bufs=3) as pa, \
             tc.tile_pool(name="pp", bufs=46) as pp, \
             tc.tile_pool(name="pb", bufs=2) as pb, \
             tc.tile_pool(name="ps_s", bufs=2, space="PSUM") as ps_s, \
             tc.tile_pool(name="ps_c", bufs=2, space="PSUM") as ps_c, \
             tc.tile_pool(name="ps_o", bufs=2, space="PSUM") as ps_o:
            ident = cp.tile([128, 128], f16, name="ident")
            make_identity(nc, ident)
            trimask = cp.tile([128, 128], f16, name="trimask")
            nc.vector.memset(trimask, 1.0)
            # keep where q - k >= 0 (upper triangle incl diag), else 0
            nc.gpsimd.affine_select(
                out=trimask, in_=trimask, compare_op=ALU.is_ge,
                fill=0.0, base=0, pattern=[[1, 128]], channel_multiplier=-1)
            epsb = cp.tile([128, 1], f32, name="epsb")
            nc.vector.memset(epsb, D * EPS)
            cos_sb = cp.tile([128, NT, HD], f16, name="cos_sb")
            sin_sb = cp.tile([128, NT, HD], f16, name="sin_sb")
            wt_sb = cp.tile([128, E], f16, name="wt_sb")
            qT = cp.tile([128, B, S], f16, name="qT")
            kT = cp.tile([128, B, S], f16, name="kT")
            vsb = cp.tile([128, B, NT, HL, D + 1], f16, name="vsb")
            rsk = cp.tile([128, B, NT, HL], f32, name="rsk")

            # ---------------- phase A: norm + rope + transpose ------------
            def phase_a(src, dstT, b, is_q, nsub=1, sq_eng="dve",
                        copy_eng="dve", rope_eng="dve", qs_eng="dve",
                        subs=None):
                NS = NT // nsub
                for sub in (subs if subs is not None else range(nsub)):
                    tsl = slice(sub * NS, (sub + 1) * NS)
                    raw = pa.tile([128, NS, HL, D], f16, tag="raw",
                                  name="raw", bufs=4,
                                  padded_shape=[128, NT, HL, D])
                    nc.sync.dma_start(out=raw, in_=src.ap()[b][:, tsl])
                    sq = pa.tile([128, NS, HL, D], f16, tag="sq", name="sq",
                                 padded_shape=[128, NT, HL, D])
                    if sq_eng == "act":
                        nc.scalar.activation(out=sq, in_=raw,
                                             func=ACTF.Square)
                    else:
                        eng = nc.gpsimd if sq_eng == "pool" else nc.vector
                        eng.tensor_mul(sq, raw, raw)
                    ss = pa.tile([128, NS, HL], f32, tag="ss", name="ss",
                                 padded_shape=[128, NT, HL])
                    nc.vector.reduce_sum(ss, sq, axis=mybir.AxisListType.X)
                    # s64 = sqrt(ss + D*eps) = sqrt(D) * rms
                    s64 = pa.tile([128, NS, HL], f32, tag="s64", name="s64",
                                  padded_shape=[128, NT, HL])
                    nc.scalar.activation(out=s64, in_=ss, func=ACTF.Sqrt,
                                         bias=epsb, scale=1.0)
                    if is_q:
                        rsq = pa.tile([128, NS, HL], f32, tag="rsq",
                                      name="rsq", padded_shape=[128, NT, HL])
                        nc.vector.reciprocal(out=rsq, in_=s64)
                        rs8 = pa.tile([128, NS, HL], f16, tag="rs8",
                                      name="rs8", padded_shape=[128, NT, HL])
                        # 1/rms = sqrt(D)/s64 -> x8
                        nc.vector.tensor_scalar_mul(rs8, rsq, 8.0)
                    else:
                        # exp scale = 1/(sqrt(D)*rms) = softmax scale / rms
                        nc.vector.reciprocal(out=rsk[:, b, tsl, :], in_=s64)
                    # rope on raw (rotation-invariant wrt the norm)
                    x1 = raw[:, :, :, 0:HD]
                    x2 = raw[:, :, :, HD:D]
                    cb = _bcast_mid(cos_sb[:, tsl], HL, 2)
                    sb = _bcast_mid(sin_sb[:, tsl], HL, 2)
                    t1 = pa.tile([128, NS, HL, HD], f16, tag="t1", name="t1",
                                 padded_shape=[128, NT, HL, HD])
                    t2 = pa.tile([128, NS, HL, HD], f16, tag="t2", name="t2",
                                 padded_shape=[128, NT, HL, HD])
                    rn = pa.tile([128, NS, HL, D], f16, tag="rn", name="rn",
                                 padded_shape=[128, NT, HL, D])
                    re_ = nc.gpsimd if rope_eng == "pool" else nc.vector
                    re_.tensor_mul(t1, x1, cb)
                    re_.tensor_mul(t2, x2, sb)
                    re_.tensor_sub(rn[:, :, :, 0:HD], t1, t2)
                    t1b = pa.tile([128, NS, HL, HD], f16, tag="t1b",
                                  name="t1b", padded_shape=[128, NT, HL, HD])
                    t2b = pa.tile([128, NS, HL, HD], f16, tag="t2b",
                                  name="t2b", padded_shape=[128, NT, HL, HD])
                    re_.tensor_mul(t1b, x1, sb)
                    re_.tensor_mul(t2b, x2, cb)
                    re_.tensor_add(rn[:, :, :, HD:D], t1b, t2b)
                    if is_q:
                        (nc.gpsimd if qs_eng == "pool" else
                         nc.vector).tensor_mul(rn, rn, _bcast(rs8, D))
                    G = NS if NS < 8 else 8
                    for g in range(NS // G):
                        quad = ps_o.tile([128, G * 128], f16, tag="o",
                                         name="quadA",
                                         padded_shape=[128, 1024])
                        for tt in range(G):
                            t = g * G + tt
                            nc.tensor.transpose(
                                quad[:, tt * 128:(tt + 1) * 128],
                                rn[:, t], ident)
                        s0 = (sub * NS + g * G) * 128
                        if copy_eng == "act":
                            nc.scalar.activation(
                                out=dstT[:, b, s0:s0 + G * 128], in_=quad,
                                func=ACTF.Copy)
                        else:
                            nc.vector.tensor_copy(
                                dstT[:, b, s0:s0 + G * 128], quad)

            def load_v(b):
                nc.sync.dma_start(out=vsb[:, b], in_=v_in.ap()[b])

            # ---------------- phase B ------------------------------------
            p_tiles = {}

            def scores(b, key, qlo, qhi, hls=(0, 1), jjs=None):
                """score matmuls + exp for chunk [qlo, qhi) of batch b."""
                for hl in hls:
                    hsl = slice(hl * D, (hl + 1) * D)
                    for jj in (jjs if jjs is not None
                               else range(qhi // 128)):
                        lo = max(jj * 128, qlo)
                        w = qhi - lo
                        sps = ps_s.tile([128, CW], f32, tag="s", name="sps")
                        off = 0
                        while off < w:
                            pw = min(512, w - off)
                            nc.tensor.matmul(
                                sps[:, off:off + pw],
                                lhsT=kT[hsl, b, jj * 128:(jj + 1) * 128],
                                rhs=qT[hsl, b, lo + off:lo + off + pw],
                                start=True, stop=True)
                            off += pw
                        pt = pp.tile([128, CW], f16, tag="p", name="pt")
                        nc.scalar.activation(
                            out=pt[:, 0:w], in_=sps[:, 0:w], func=ACTF.Exp,
                            scale=rsk[:, b, jj, hl:hl + 1])
                        if lo == jj * 128:
                            dg = nc.gpsimd if DIAG_ON_POOL else nc.vector
                            dg.tensor_mul(pt[:, 0:128], pt[:, 0:128],
                                          trimask)
                        p_tiles[(b, key, hl, jj)] = pt
                        pump(PUMP_K)

            ctx_feed = []

            def pump(n):
                """Drain up to n queued ctx matmuls (norm ops ride free)."""
                while n > 0 and ctx_feed:
                    kind, fn = ctx_feed.pop(0)
                    fn()
                    if kind == "mm":
                        n -= 1

            def drain():
                pump(1 << 30)

            def ctx_part(b, key, qlo, igs):
                """Queue ctx matmuls + rowsum-norm for global q-tiles `igs`
                (contiguous, <= 4); drained by pump() inside later scores
                so PE score matmuls keep feeding ACT between ctx bursts."""
                ng = len(igs)
                cpr = pb.tile([128, ng, HL, D], f16, tag="cp", bufs=4,
                              name="cpr", padded_shape=[128, 4, HL, D])
                for hl in range(HL):
                    ctx = ps_c.tile([128, ng, D + 1], f32, tag="c",
                                    name="ctx", padded_shape=[128, 4, D + 1])
                    for gi, ig in enumerate(igs):
                        for jj in range(ig + 1):
                            def mm(hl=hl, ctx=ctx, gi=gi, ig=ig, jj=jj):
                                pt = p_tiles[(b, key, hl, jj)]
                                lo = max(jj * 128, qlo)
                                col = ig * 128 - lo
                                nc.tensor.matmul(
                                    ctx[:, gi, :],
                                    lhsT=pt[:, col:col + 128],
                                    rhs=vsb[:, b, jj, hl, :],
                                    start=(jj == 0), stop=(jj == ig))
                            ctx_feed.append(("mm", mm))

                    def norm(hl=hl, ctx=ctx, cpr=cpr):
                        rsr = pb.tile([128, ctx.shape[1]], f32, tag="rsr",
                                      bufs=4, name="rsr",
                                      padded_shape=[128, 4])
                        nc.vector.reciprocal(out=rsr, in_=ctx[:, :, D])
                        nc.vector.tensor_mul(
                            cpr[:, :, hl, :], ctx[:, :, 0:D], _bcast(rsr, D))
                    ctx_feed.append(("end", norm))
                done = {"v": False}
                ctx_feed.append(("end", lambda: done.update(v=True)))
                return (b, igs, cpr, done)

            def proj_part(info, act_copies=False):
                b, igs, cpr, done = info
                ng = len(igs)
                while not done["v"]:
                    pump(1)
                quad = ps_o.tile([128, ng * 128], f16, tag="o", name="quadT",
                                 padded_shape=[128, 1024])
                for gi in range(ng):
                    nc.tensor.transpose(
                        quad[:, gi * 128:(gi + 1) * 128], cpr[:, gi], ident)
                ctxT = pb.tile([128, ng, 128], f16, tag="ctxT", bufs=3,
                               name="ctxT", padded_shape=[128, 4, 128])
                nc.vector.tensor_copy(ctxT, quad)
                for gi, ig in enumerate(igs):
                    osb = pb.tile([128, E], f16, tag="osb", bufs=5,
                                  name="osb")
                    for nn in range(2):
                        po = ps_o.tile([128, 512], f32, tag="o", name="po")
                        nc.tensor.matmul(
                            po, lhsT=ctxT[:, gi],
                            rhs=wt_sb[:, nn * 512:(nn + 1) * 512],
                            start=True, stop=True)
                        on_act = (act_copies and nn == 0) or \
                            OSB_ACT_PATTERN[(ig * 2 + nn)
                                            % len(OSB_ACT_PATTERN)]
                        if on_act:
                            nc.scalar.activation(
                                out=osb[:, nn * 512:(nn + 1) * 512],
                                in_=po, func=ACTF.Copy)
                        else:
                            nc.vector.tensor_copy(
                                osb[:, nn * 512:(nn + 1) * 512], po)
                    row0 = b * S + ig * 128
                    nc.sync.dma_start(out=out.ap()[row0:row0 + 128, :],
                                      in_=osb)

            # ---------------- schedule -----------------------------------
            # Software pipeline: proj of group N-1 is emitted after ctx of
            # group N so DVE's osb copies overlap PE's next ctx matmuls.
            # Phase A is emitted per half (sub) so the first scores start
            # as soon as the first halves of kT/qT are transposed.
            pend = []

            def ctxg(b, key, qlo, igs):
                pend.append(ctx_part(b, key, qlo, igs))

            def projg(n=1, act_copies=False):
                for _ in range(n):
                    proj_part(pend.pop(0), act_copies=act_copies)

            # b0 chunks A=(0,512) B=(512,1024) C=(1024,2048);
            # b1 chunks D=(0,1024) E=(1024,2048)
            nc.sync.dma_start(out=cos_sb, in_=cos_in.ap())
            nc.sync.dma_start(out=sin_sb, in_=sin_in.ap())
            phase_a(k_in, kT, 0, False, nsub=4, subs=[0], copy_eng="act",
                    sq_eng="act")
            phase_a(q_in, qT, 0, True, nsub=4, subs=[0], copy_eng="act",
                    sq_eng="act")
            nc.sync.dma_start(out=wt_sb, in_=wt_in.ap())
            load_v(0)
            scores(0, "A", 0, 512)
            phase_a(k_in, kT, 0, False, nsub=4, subs=[1], copy_eng="act",
                    sq_eng="act")
            phase_a(q_in, qT, 0, True, nsub=4, subs=[1], copy_eng="act",
                    sq_eng="act")
            scores(0, "B", 512, 1024)
            ctxg(0, "A", 0, [0, 1])
            phase_a(q_in, qT, 0, True, nsub=4, subs=[2, 3], sq_eng="act")
            ctxg(0, "A", 0, [2, 3])
            scores(0, "C", 1024, 2048, hls=(0,), jjs=range(0, 8))
            phase_a(k_in, kT, 0, False, nsub=4, subs=[2, 3], sq_eng="act")
            scores(0, "C", 1024, 2048, hls=(1,), jjs=range(0, 8))
            ctxg(0, "B", 512, [4, 5])
            projg()
            scores(0, "C", 1024, 2048, hls=(0,), jjs=range(8, 16))
            projg(0)
            phase_a(k_in, kT, 1, False, nsub=2, subs=[0], sq_eng="pool")
            phase_a(q_in, qT, 1, True, nsub=2, subs=[0], sq_eng="pool")
            load_v(1)
            phase_a(k_in, kT, 1, False, nsub=2, subs=[1], sq_eng="pool")
            phase_a(q_in, qT, 1, True, nsub=2, subs=[1], sq_eng="pool")
            scores(0, "C", 1024, 2048, hls=(1,), jjs=range(8, 16))
            ctxg(0, "B", 512, [6, 7])
            projg()
            scores(1, "D", 0, 1024, hls=(0,))
            ctxg(0, "C", 1024, [8, 9])
            projg()
            ctxg(0, "C", 1024, [10, 11])
            projg()
            scores(1, "D", 0, 1024, hls=(1,))
            ctxg(0, "C", 1024, [12, 13])
            projg()
            ctxg(0, "C", 1024, [14, 15])
            projg()
            scores(1, "E", 1024, 2048, jjs=range(0, 4))
            ctxg(1, "D", 0, [0, 1])
            projg()
            scores(1, "E", 1024, 2048, jjs=range(4, 8))
            ctxg(1, "D", 0, [2, 3])
            projg()
            scores(1, "E", 1024, 2048, jjs=range(8, 10))
            ctxg(1, "D", 0, [4, 5])
            projg()
            scores(1, "E", 1024, 2048, jjs=range(10, 12))
            ctxg(1, "E", 1024, [8, 9])
            projg()
            ctxg(1, "D", 0, [6, 7])
            projg()
            scores(1, "E", 1024, 2048, jjs=range(12, 14))
            ctxg(1, "E", 1024, [10, 11])
            projg()
            ctxg(1, "E", 1024, [12, 13])
            projg(1, act_copies=True)
            scores(1, "E", 1024, 2048, jjs=range(14, 15))
            ctxg(1, "E", 1024, [14])
            drain()
            projg(1, act_copies=True)
            scores(1, "E", 1024, 2048, jjs=range(15, 16))
            ctxg(1, "E", 1024, [15])
            drain()
            while pend:
                projg(1, act_copies=True)
    _legalize_waits(nc)
    return nc


# even rope lanes first, then odd (consistent perm leaves q.k unchanged)
_ROPE_PERM = np.concatenate([np.arange(0, D, 2), np.arange(1, D, 2)])


def _shard_inputs(q, k, v, cos, sin, proj_w):
    """Per-core input maps (host-side layout/dtype prep only)."""
    qh = q.reshape(B, S, H, D)
    kh = k.reshape(B, S, H, D)
    vh = v.reshape(B, S, H, D)
    cos_t = np.ascontiguousarray(
        cos.reshape(NT, 128, HD).transpose(1, 0, 2), np.float16)
    sin_t = np.ascontiguousarray(
        sin.reshape(NT, 128, HD).transpose(1, 0, 2), np.float16)
    maps = []
    for core in range(N_CORES):
        hs = slice(HL * core, HL * (core + 1))

        def pack_qk(x):
            xs = x[:, :, hs, :][..., _ROPE_PERM]      # [B, S, HL, D]
            xs = xs.reshape(B, NT, 128, HL * D)
            return np.ascontiguousarray(
                xs.transpose(0, 2, 1, 3), np.float16)  # [B, 128, NT, D2]

        vs = vh[:, :, hs, :]                           # [B, S, HL, D]
        vcat = np.concatenate(
            [vs, np.ones((B, S, HL, 1), vs.dtype)], axis=-1)
        v_map = np.ascontiguousarray(
            vcat.reshape(B, NT, 128, HL, D + 1).transpose(0, 2, 1, 3, 4),
            np.float16)                                # [B, 128, NT, HL, 65]
        wt_c = np.ascontiguousarray(
            proj_w[:, 128 * core:128 * (core + 1)].T, np.float16)
        maps.append({
            "q": pack_qk(qh), "k": pack_qk(kh), "v": v_map,
            "cos": cos_t, "sin": sin_t, "wt": wt_c,
        })
    return maps


_NC_CACHE = []


def _get_nc():
    if not _NC_CACHE:
        _NC_CACHE.append(build_nc())
    return _NC_CACHE[0]


def kernel(q, k, v, attn_mask, padding_mask, qn_w, kn_w, proj_w, proj_b,
           cos, sin):
    q = np.asarray(q, np.float32)
    k = np.asarray(k, np.float32)
    v = np.asarray(v, np.float32)
    proj_w = np.asarray(proj_w, np.float32)
    proj_b = np.asarray(proj_b, np.float32)
    cos = np.asarray(cos, np.float32)
    sin = np.asarray(sin, np.float32)
    attn_mask = np.asarray(attn_mask)
    padding_mask = np.asarray(padding_mask)
    qn_w = np.asarray(qn_w, np.float32)
    kn_w = np.asarray(kn_w, np.float32)
    # The kernel bakes in: causal attn_mask, no padding, unit RMSNorm weights.
    assert np.array_equal(
        attn_mask.reshape(S, S), np.tril(np.ones((S, S), attn_mask.dtype)))
    assert padding_mask.all()
    assert np.all(qn_w == 1.0) and np.all(kn_w == 1.0)

    in_maps = _shard_inputs(q, k, v, cos, sin, proj_w)
    nc = _get_nc()
    # Retry guard: the tunneled device occasionally throws transient
    # INTERNAL errors or (rarely) returns a bad first launch.  Verify by
    # re-running core 0 alone and checking it reproduces its partial.
    parts = None
    for attempt in range(4):
        try:
            res = run_bass_kernel_spmd(nc, in_maps,
                                       core_ids=list(range(N_CORES)))
            cand = np.stack([r["out"] for r in res.results])  # [8, B*S, E]
            chk = run_bass_kernel_spmd(nc, in_maps[:1], core_ids=[0])
            chk0 = np.asarray(chk.results[0]["out"])
        except Exception:
            if attempt == 3:
                raise
            import time
            time.sleep(3)
            continue
        parts = cand
        if np.array_equal(chk0, cand[0]):
            break
    full = parts.astype(np.float32).sum(axis=0) + proj_b[None, :]
    return full.reshape(B, S, E).astype(np.float32)
